# revision 3
# baseline (speedup 1.0000x reference)
"""Cross-attention (single-head, residual) Bass/Tile kernel for Trainium2.

Problem: y = x + (softmax((x' Wq + bq)(ctx Wk + bk)^T / sqrt(C)) (ctx Wv + bv)) Wo + bo
  x: [B=8, C=512, H=64, W=64], context: [B=8, Lc=512, CTX=768]

Sharding: pure data-parallel over batch - one batch element per NeuronCore,
no collectives.

Algebraic restructuring (saves ~1/3 of the matmul work): with
  kT = (ctx Wk + bk)^T           [C, Lc]
  G  = Wq kT                     [C, Lc]   (Wq folded into the key side)
  vW = (ctx Wv + bv) Wo + 1 bo^T [Lc, C]   (Wo and bo folded into the value side)
the streaming loop per hw-tile is two matmul stages:
  simT = G^T-contracted-with-x:  simT[lc,hw] = sum_c' x[c',hw] G[c',lc]
  eT   = exp(scale*simT + scale*(kT^T bq))        (bq folded into ACT bias)
  eN   = eT * (64/colsum(eT))                     (pre-normalized, x64 for fp8 range)
  y    = (vW^T eN) / 64 + x                       (single fused eviction op)
bv/bo are exact under the fold because softmax rows sum to 1.

Host-side prep (layout/dtype only, no math): x ships twice - bf16 for the
residual and fp8e4 as the sim moving operand; ctx arrives pre-transposed in
fp8; Wq pre-transposed; weights fp8 scaled by 32 (raw std 0.02 is fp8
subnormal; evictions unscale).  PE runs fp8 DoubleRow throughout.  A block
of dummy warm-up matmuls fills the initial DMA wait so the PE HAM clock
gate opens (1.2 -> 2.4 GHz) before real work arrives.  Output y is bf16.
"""

import numpy as np
import ml_dtypes

B = 8
C = 512
CTX = 768
Lc = 512
HH = 64
WW = 64
HW = HH * WW          # 4096
N_CORES = 8
P = 128
HT = 512              # hw tile (free-dim) width
N_HT = HW // HT       # 8
KC = C // P           # 4
KX = CTX // P         # 6
KL = Lc // P          # 4
SCALE = float(C) ** -0.5
WS = 32.0             # host-side fp8 weight scaling
ES = 64.0             # eT normalization headroom scale
N_WARM = 28           # PE warm-up matmuls during the initial DMA wait

NP_BF16 = ml_dtypes.bfloat16
NP_FP8 = ml_dtypes.float8_e4m3

_cache = {}


def _build_nc():
    import concourse.mybir as mybir
    import concourse.bass as bass
    import concourse.tile as tile
    from concourse import bacc

    f32 = mybir.dt.float32
    bf16 = mybir.dt.bfloat16
    fp8 = mybir.dt.float8e4
    AF = mybir.ActivationFunctionType
    ALU = mybir.AluOpType
    DR = mybir.MatmulPerfMode.DoubleRow

    nc = bacc.Bacc("TRN2", target_bir_lowering=False, debug=False,
                   num_devices=N_CORES)

    x16_d = nc.dram_tensor("x16", [C, HW], bf16, kind="ExternalInput").ap()
    x8_d = nc.dram_tensor("x8", [C, HW], fp8, kind="ExternalInput").ap()
    ctxT_d = nc.dram_tensor("ctxT8", [CTX, Lc], fp8, kind="ExternalInput").ap()
    wk_d = nc.dram_tensor("wk8", [CTX, C], fp8, kind="ExternalInput").ap()
    wv_d = nc.dram_tensor("wv8", [CTX, C], fp8, kind="ExternalInput").ap()
    wqT_d = nc.dram_tensor("wqT8", [C, C], fp8, kind="ExternalInput").ap()
    wo_d = nc.dram_tensor("wo8", [C, C], fp8, kind="ExternalInput").ap()
    bq_d = nc.dram_tensor("bq8", [C], fp8, kind="ExternalInput").ap()
    bk_d = nc.dram_tensor("bk", [C], f32, kind="ExternalInput").ap()
    bv_d = nc.dram_tensor("bv", [C], f32, kind="ExternalInput").ap()
    bo_d = nc.dram_tensor("bo", [C], f32, kind="ExternalInput").ap()
    y_d = nc.dram_tensor("y", [C, HW], bf16, kind="ExternalOutput").ap()

    x16_r = x16_d.rearrange("(ko p) hw -> p ko hw", p=P)    # [128, 4, 4096]
    x8_r = x8_d.rearrange("(ko p) hw -> p ko hw", p=P)
    y_r = y_d.rearrange("(ko p) hw -> p ko hw", p=P)
    ctxT_r = ctxT_d.rearrange("(ko p) lc -> p ko lc", p=P)  # [128, 6, 512]
    wk_r = wk_d.rearrange("(ko p) c -> p ko c", p=P)        # [128, 6, 512]
    wv_r = wv_d.rearrange("(ko p) c -> p ko c", p=P)
    wqT_r = wqT_d.rearrange("(ko p) c -> p ko c", p=P)      # [128, 4, 512]
    wo_r = wo_d.rearrange("(ko p) c -> p ko c", p=P)

    with tile.TileContext(nc) as tc:
        with (
            tc.tile_pool(name="const", bufs=1) as const,
            tc.tile_pool(name="xin", bufs=5) as xin,
            tc.tile_pool(name="xin8", bufs=4) as xin8,
            tc.tile_pool(name="work", bufs=3) as work,
            tc.tile_pool(name="yout", bufs=2) as yout,
            tc.tile_pool(name="small", bufs=3) as small,
            tc.tile_pool(name="psum", bufs=3, space="PSUM") as psum,
            tc.tile_pool(name="psum_s", bufs=1, space="PSUM") as psum_s,
            tc.tile_pool(name="psum_bc", bufs=1, space="PSUM") as psum_bc,
        ):
            # ---------------- DMAs (ordered by when the PE needs them) -----
            ctx_f = const.tile([P, KX, Lc], fp8, name="ctx_f", tag="ctx_f")
            wk_f = const.tile([P, KX, C], fp8, name="wk_f", tag="wk_f")
            for u in range(KX // 2):
                cs = slice(2 * u, 2 * u + 2)
                nc.sync.dma_start(out=ctx_f[:, cs, :], in_=ctxT_r[:, cs, :])
                nc.sync.dma_start(out=wk_f[:, cs, :], in_=wk_r[:, cs, :])
            wqT_f = const.tile([P, KC, C], fp8, name="wqT_f", tag="wqT_f")
            nc.sync.dma_start(out=wqT_f, in_=wqT_r)

            x16_t, x8_t = {}, {}

            def fetch(h):
                if h < N_HT and h not in x8_t:
                    hs = slice(h * HT, (h + 1) * HT)
                    t8 = xin8.tile([P, KC, HT], fp8, tag="x8", name=f"x8_{h}")
                    nc.sync.dma_start(out=t8, in_=x8_r[:, :, hs])
                    x8_t[h] = t8
                    t16 = xin.tile([P, KC, HT], bf16, tag="x16",
                                   name=f"x16_{h}")
                    nc.sync.dma_start(out=t16, in_=x16_r[:, :, hs])
                    x16_t[h] = t16

            fetch(0)
            wv_f = const.tile([P, KX, C], fp8, name="wv_f", tag="wv_f")
            nc.sync.dma_start(out=wv_f, in_=wv_r)
            wo_f = const.tile([P, KC, C], fp8, name="wo_f", tag="wo_f")
            nc.sync.dma_start(out=wo_f, in_=wo_r)
            fetch(1)
            fetch(2)

            # constants for the softmax-denominator matvec/broadcast
            ones2 = const.tile([P, 2, 16], fp8, name="ones2", tag="ones2")
            nc.vector.memset(ones2, 1.0)
            # broadcast row carries the 1/ES fold: ps_bc = colsum/ES
            ones_row = const.tile([1, P], bf16, name="ones_row", tag="ones_r")
            nc.vector.memset(ones_row, 1.0 / ES)

            # PE warm-up: dummy matmuls on a memset tile fill the initial
            # DMA wait so the HAM clock gate opens before real work arrives
            warm_sb = const.tile([P, HT], bf16, name="warm_sb", tag="warm")
            nc.vector.memset(warm_sb, 0.0)
            ps_w = psum_bc.tile([P, HT], f32, tag="bc", name="ps_warm")
            for w in range(N_WARM):
                nc.tensor.matmul(ps_w, warm_sb[:, :P], warm_sb,
                                 start=True, stop=True)

            # biases (tiny scattered DMAs on the gpsimd queue)
            bq_t = const.tile([P, KC], fp8, name="bq_t", tag="bq")
            bk_t = const.tile([P, KC], f32, name="bk_t", tag="bk")
            bv_t = const.tile([P, KC], f32, name="bv_t", tag="bv")
            with nc.allow_non_contiguous_dma(reason="tiny one-time bias loads"):
                nc.gpsimd.dma_start(out=bq_t, in_=bq_d.rearrange("(ko p) -> p ko", p=P))
                nc.gpsimd.dma_start(out=bk_t, in_=bk_d.rearrange("(ko p) -> p ko", p=P))
                nc.gpsimd.dma_start(out=bv_t, in_=bv_d.rearrange("(ko p) -> p ko", p=P))
            # bo broadcast across partitions (folded into vW exactly)
            bo_bc = const.tile([P, C], f32, name="bo_bc", tag="bo")
            bo_src = bass.AP(tensor=bo_d.tensor, offset=bo_d.offset,
                             ap=[[0, P]] + list(bo_d.ap))
            nc.gpsimd.dma_start(out=bo_bc, in_=bo_src)

            # ---------------- phase A (all fp8 DoubleRow, no transposes) ---
            # kT [128(c), KC, Lc] = (ctx Wk + bk)^T
            kT_8 = const.tile([P, KC, Lc], fp8, name="kT_8", tag="kT")
            for mc in range(KC):
                ps = psum.tile([P, Lc], f32, tag="mm", name=f"ps_k_{mc}")
                for u in range(KX // 2):
                    nc.tensor.matmul(ps,
                                     wk_f[:, 2 * u:2 * u + 2,
                                          mc * P:(mc + 1) * P],
                                     ctx_f[:, 2 * u:2 * u + 2, :],
                                     start=(u == 0), stop=(u == KX // 2 - 1),
                                     perf_mode=DR)
                nc.scalar.activation(kT_8[:, mc, :], ps, AF.Identity,
                                     scale=1.0 / WS, bias=bk_t[:, mc:mc + 1])

            # G [128(c'), KC, Lc] = Wq kT
            G_8 = const.tile([P, KC, Lc], fp8, name="G_8", tag="G")
            for mg in range(KC):
                ps = psum.tile([P, Lc], f32, tag="mmy", name=f"ps_g_{mg}")
                for u in range(KC // 2):
                    nc.tensor.matmul(ps,
                                     wqT_f[:, 2 * u:2 * u + 2,
                                           mg * P:(mg + 1) * P],
                                     kT_8[:, 2 * u:2 * u + 2, :],
                                     start=(u == 0), stop=(u == KC // 2 - 1),
                                     perf_mode=DR)
                nc.scalar.activation(G_8[:, mg, :], ps, AF.Copy,
                                     scale=1.0 / WS)

            # bqk_s [128(lc), KL] = SCALE * kT^T bq   (per-lc exp bias)
            bqk_s = const.tile([P, KL], f32, name="bqk_s", tag="bqk")
            for ml in range(KL):
                ps = psum.tile([P, HT], f32, tag="mm", name=f"ps_bq_{ml}")
                for mc in range(KC):
                    nc.tensor.matmul(ps[:, 0:1],
                                     kT_8[:, mc, ml * P:(ml + 1) * P],
                                     bq_t[:, mc:mc + 1],
                                     start=(mc == 0), stop=(mc == KC - 1))
                nc.scalar.activation(bqk_s[:, ml:ml + 1], ps[:, 0:1],
                                     AF.Identity, scale=SCALE / WS)

            # vT [128(c), KC, Lc] = (ctx Wv + bv)^T
            vT_8 = const.tile([P, KC, Lc], fp8, name="vT_8", tag="vT")
            for mc in range(KC):
                ps = psum.tile([P, Lc], f32, tag="mm", name=f"ps_vt_{mc}")
                for u in range(KX // 2):
                    nc.tensor.matmul(ps,
                                     wv_f[:, 2 * u:2 * u + 2,
                                          mc * P:(mc + 1) * P],
                                     ctx_f[:, 2 * u:2 * u + 2, :],
                                     start=(u == 0), stop=(u == KX // 2 - 1),
                                     perf_mode=DR)
                nc.scalar.activation(vT_8[:, mc, :], ps, AF.Identity,
                                     scale=1.0 / WS, bias=bv_t[:, mc:mc + 1])

            # vW [128(lc), KL, C(c_out)] = (v + bv) Wo + 1 bo^T
            vW_8 = const.tile([P, KL, C], fp8, name="vW_8", tag="vW")
            for ml in range(KL):
                ps = psum.tile([P, C], f32, tag="mmy", name=f"ps_vw_{ml}")
                for u in range(KC // 2):
                    nc.tensor.matmul(ps,
                                     vT_8[:, 2 * u:2 * u + 2,
                                          ml * P:(ml + 1) * P],
                                     wo_f[:, 2 * u:2 * u + 2, :],
                                     start=(u == 0), stop=(u == KC // 2 - 1),
                                     perf_mode=DR)
                nc.vector.scalar_tensor_tensor(
                    out=vW_8[:, ml, :], in0=ps, scalar=1.0 / WS, in1=bo_bc,
                    op0=ALU.mult, op1=ALU.add)

            # ---------------- phase B: stream over hw tiles ----------------
            def emit_yT(h, eN, last=False):
                # y [c_out, hw] = (vW^T eN)/ES + x  (fused eviction)
                x16 = x16_t[h]
                y_sb = yout.tile([P, KC, HT], bf16, tag="y", name=f"y_{h}")
                hs = slice(h * HT, (h + 1) * HT)
                for mo in range(KC):
                    ps = psum.tile([P, HT], f32, tag="mmy",
                                   name=f"ps_y_{h}_{mo}")
                    for u in range(KL // 2):
                        nc.tensor.matmul(ps,
                                         vW_8[:, 2 * u:2 * u + 2,
                                              mo * P:(mo + 1) * P],
                                         eN[:, 2 * u:2 * u + 2, :],
                                         start=(u == 0), stop=(u == KL // 2 - 1),
                                         perf_mode=DR)
                    # y = ps/ES + x in one DVE op (normalization pre-applied
                    # to eN, so the eviction is mul-by-imm + residual add)
                    nc.vector.scalar_tensor_tensor(
                        out=y_sb[:, mo, :], in0=ps, scalar=1.0 / ES,
                        in1=x16[:, mo, :], op0=ALU.mult, op1=ALU.add)
                    if last:
                        nc.sync.dma_start(out=y_r[:, mo, hs],
                                          in_=y_sb[:, mo, :])
                if not last:
                    nc.sync.dma_start(out=y_r[:, :, hs], in_=y_sb)

            prev = [None, None]
            for h in range(N_HT):
                fetch(h + 3)
                x_8 = x8_t[h]

                # eT [lc, hw] = exp(scale * (G^T x) + scale * kT^T bq)
                eT = work.tile([P, KL, HT], fp8, tag="eT", name=f"eT_{h}")
                for ml in range(KL):
                    ps = psum.tile([P, HT], f32, tag="mm", name=f"ps_s_{h}_{ml}")
                    for u in range(KC // 2):
                        nc.tensor.matmul(ps,
                                         G_8[:, 2 * u:2 * u + 2,
                                             ml * P:(ml + 1) * P],
                                         x_8[:, 2 * u:2 * u + 2, :],
                                         start=(u == 0), stop=(u == KC // 2 - 1),
                                         perf_mode=DR)
                    nc.scalar.activation(eT[:, ml, :], ps, AF.Exp, scale=SCALE,
                                         bias=bqk_s[:, ml:ml + 1])

                # attn@V runs two tiles behind so the normalized eN is ready
                # before its matmuls reach the head of the PE queue
                if prev[0] is not None:
                    emit_yT(*prev[0])
                prev[0] = prev[1]

                # softmax denominator: ones^T @ eT (DoubleRow matvec pairs)
                ps_sum = psum_s.tile([1, HT], f32, tag="sum", name=f"ps_sum_{h}")
                for u in range(KL // 2):
                    nc.tensor.matmul(ps_sum, ones2[:, :, 0:1],
                                     eT[:, 2 * u:2 * u + 2, :],
                                     start=(u == 0), stop=(u == KL // 2 - 1),
                                     perf_mode=DR)
                sum_sb = small.tile([1, HT], bf16, tag="sum_sb",
                                    name=f"sum_sb_{h}")
                nc.scalar.activation(sum_sb, ps_sum, AF.Copy)

                ps_bc = psum_bc.tile([P, HT], f32, tag="bc", name=f"ps_bc_{h}")
                nc.tensor.matmul(ps_bc, ones_row, sum_sb, start=True, stop=True)
                rec_sb = work.tile([P, HT], f32, tag="rec_sb",
                                   name=f"rec_sb_{h}")
                nc.vector.reciprocal_approx_fast(out=rec_sb, in_=ps_bc)
                rec16 = work.tile([P, HT], bf16, tag="rec16",
                                  name=f"rec16_{h}")
                nc.scalar.activation(rec16, rec_sb, AF.Copy)

                # eN = eT * (ES/colsum)  [3 chunks gpsimd, 1 chunk DVE]
                eN = work.tile([P, KL, HT], fp8, tag="eN", name=f"eN_{h}")
                for ml in range(KL):
                    eng = nc.vector if ml == 3 else nc.gpsimd
                    eng.tensor_mul(out=eN[:, ml, :], in0=eT[:, ml, :],
                                   in1=rec16)
                prev[1] = (h, eN)

            emit_yT(*prev[0])
            emit_yT(*prev[1], last=True)

    nc.compile()
    return nc


def _get_compiled():
    if "nc" not in _cache:
        _cache["nc"] = _build_nc()
    return _cache["nc"]


def _make_in_maps(x, context, Wq, bq, Wk, bk, Wv, bv, Wo, bo):
    x = np.asarray(x, dtype=np.float32)
    context = np.asarray(context, dtype=np.float32)
    common = {
        "wk8": np.ascontiguousarray((np.asarray(Wk, np.float32) * WS).astype(NP_FP8)),
        "wv8": np.ascontiguousarray((np.asarray(Wv, np.float32) * WS).astype(NP_FP8)),
        "wqT8": np.ascontiguousarray((np.asarray(Wq, np.float32).T * WS).astype(NP_FP8)),
        "wo8": np.ascontiguousarray((np.asarray(Wo, np.float32) * WS).astype(NP_FP8)),
        "bq8": np.ascontiguousarray((np.asarray(bq, np.float32) * WS).astype(NP_FP8)),
        "bk": np.ascontiguousarray(np.asarray(bk, dtype=np.float32)),
        "bv": np.ascontiguousarray(np.asarray(bv, dtype=np.float32)),
        "bo": np.ascontiguousarray(np.asarray(bo, dtype=np.float32)),
    }
    in_maps = []
    for b in range(B):
        m = dict(common)
        xb = x[b].reshape(C, HW)
        m["x16"] = np.ascontiguousarray(xb.astype(NP_BF16))
        m["x8"] = np.ascontiguousarray(xb.astype(NP_FP8))
        m["ctxT8"] = np.ascontiguousarray(context[b].T.astype(NP_FP8))
        in_maps.append(m)
    return in_maps


def _run(in_maps, trace=False):
    from concourse.bass_utils import run_bass_kernel_spmd
    nc = _get_compiled()
    return run_bass_kernel_spmd(nc, in_maps, core_ids=list(range(N_CORES)),
                                trace=trace)


def kernel(x, context, Wq, bq, Wk, bk, Wv, bv, Wo, bo):
    in_maps = _make_in_maps(x, context, Wq, bq, Wk, bk, Wv, bv, Wo, bo)
    res = _run(in_maps, trace=False)
    out = np.stack([np.asarray(res.results[b]["y"], dtype=np.float32)
                    .reshape(C, HH, WW) for b in range(B)])
    return out


# revision 5
# speedup vs baseline: 1.0836x; 1.0836x over previous
"""Cross-attention (single-head, residual) Bass/Tile kernel for Trainium2.

Problem: y = x + (softmax((x' Wq + bq)(ctx Wk + bk)^T / sqrt(C)) (ctx Wv + bv)) Wo + bo
  x: [B=8, C=512, H=64, W=64], context: [B=8, Lc=512, CTX=768]

Sharding: pure data-parallel over batch - one batch element per NeuronCore,
no collectives.

Algebraic restructuring (saves ~1/3 of the matmul work): with
  kT = (ctx Wk + bk)^T           [C, Lc]
  G  = Wq kT                     [C, Lc]   (Wq folded into the key side)
  vW = (ctx Wv + bv) Wo + 1 bo^T [Lc, C]   (Wo and bo folded into the value side)
the streaming loop per hw-tile is two matmul stages:
  simT[lc,hw] = sum_c' x[c',hw] G[c',lc]           (8 fp8 DoubleRow matmuls)
  eT   = exp(scale*simT + scale*(kT^T bq))         (bq folded into ACT bias)
  yT[hw,c] = (eT^T vW) * (1/colsum eT) + x^T       (eT is the STATIONARY)
bv/bo are exact under the fold because softmax rows sum to 1.

The attn@V matmul runs transposed (output partitions = hw) so the softmax
denominator is a per-partition scalar: colsum rides the same stationary as
free-dim-1 matmuls into a per-hw psum column, one reciprocal per tile, and
normalization + residual add fuse into a single scalar_tensor_tensor
eviction.  No broadcast matmul, no cross-engine normalization chain.

Host-side prep (layout/dtype only, no math): x ships twice - transposed
bf16 for the residual and fp8e4 as the sim moving operand; ctx arrives
pre-transposed in fp8; Wq pre-transposed; weights fp8 scaled by 32 (raw
std 0.02 is fp8 subnormal; evictions unscale).  A block of dummy warm-up
matmuls fills the initial DMA wait so the PE HAM clock gate opens
(1.2 -> 2.4 GHz) before real work arrives.  y is written [HW, C] bf16 and
transposed back on the host.
"""

import numpy as np
import ml_dtypes

B = 8
C = 512
CTX = 768
Lc = 512
HH = 64
WW = 64
HW = HH * WW          # 4096
N_CORES = 8
P = 128
HT = 512              # hw tile (free-dim) width
N_HT = HW // HT       # 8
NCH = HT // P         # 4 hw chunks per tile
KC = C // P           # 4
KX = CTX // P         # 6
KL = Lc // P          # 4
SCALE = float(C) ** -0.5
WS = 32.0             # host-side fp8 weight scaling
N_WARM = 28           # PE warm-up matmuls during the initial DMA wait

NP_BF16 = ml_dtypes.bfloat16
NP_FP8 = ml_dtypes.float8_e4m3

_cache = {}


def _build_nc():
    import concourse.mybir as mybir
    import concourse.bass as bass
    import concourse.tile as tile
    from concourse import bacc

    f32 = mybir.dt.float32
    bf16 = mybir.dt.bfloat16
    fp8 = mybir.dt.float8e4
    AF = mybir.ActivationFunctionType
    ALU = mybir.AluOpType
    DR = mybir.MatmulPerfMode.DoubleRow

    nc = bacc.Bacc("TRN2", target_bir_lowering=False, debug=False,
                   num_devices=N_CORES)

    xT_d = nc.dram_tensor("xT16", [HW, C], bf16, kind="ExternalInput").ap()
    x8_d = nc.dram_tensor("x8", [C, HW], fp8, kind="ExternalInput").ap()
    ctxT_d = nc.dram_tensor("ctxT8", [CTX, Lc], fp8, kind="ExternalInput").ap()
    wk_d = nc.dram_tensor("wk8", [CTX, C], fp8, kind="ExternalInput").ap()
    wv_d = nc.dram_tensor("wv8", [CTX, C], fp8, kind="ExternalInput").ap()
    wqT_d = nc.dram_tensor("wqT8", [C, C], fp8, kind="ExternalInput").ap()
    wo_d = nc.dram_tensor("wo8", [C, C], fp8, kind="ExternalInput").ap()
    bq_d = nc.dram_tensor("bq8", [C], fp8, kind="ExternalInput").ap()
    bk_d = nc.dram_tensor("bk", [C], f32, kind="ExternalInput").ap()
    bv_d = nc.dram_tensor("bv", [C], f32, kind="ExternalInput").ap()
    bo_d = nc.dram_tensor("bo", [C], f32, kind="ExternalInput").ap()
    y_d = nc.dram_tensor("yT", [HW, C], bf16, kind="ExternalOutput").ap()

    xT_r = xT_d.rearrange("(hh p) c -> p hh c", p=P)        # [128, 32, 512]
    y_r = y_d.rearrange("(hh p) c -> p hh c", p=P)
    x8_r = x8_d.rearrange("(ko p) hw -> p ko hw", p=P)      # [128, 4, 4096]
    ctxT_r = ctxT_d.rearrange("(ko p) lc -> p ko lc", p=P)  # [128, 6, 512]
    wk_r = wk_d.rearrange("(ko p) c -> p ko c", p=P)        # [128, 6, 512]
    wv_r = wv_d.rearrange("(ko p) c -> p ko c", p=P)
    wqT_r = wqT_d.rearrange("(ko p) c -> p ko c", p=P)      # [128, 4, 512]
    wo_r = wo_d.rearrange("(ko p) c -> p ko c", p=P)

    with tile.TileContext(nc) as tc:
        with (
            tc.tile_pool(name="const", bufs=1) as const,
            tc.tile_pool(name="xin", bufs=4) as xin,
            tc.tile_pool(name="xin8", bufs=4) as xin8,
            tc.tile_pool(name="work", bufs=3) as work,
            tc.tile_pool(name="yout", bufs=2) as yout,
            tc.tile_pool(name="small", bufs=3) as small,
            tc.tile_pool(name="psum", bufs=3, space="PSUM") as psum,
            tc.tile_pool(name="psum_st", bufs=2, space="PSUM") as psum_st,
        ):
            # ---------------- DMAs (ordered by when the PE needs them) -----
            ctx_f = const.tile([P, KX, Lc], fp8, name="ctx_f", tag="ctx_f")
            wk_f = const.tile([P, KX, C], fp8, name="wk_f", tag="wk_f")
            for u in range(KX // 2):
                cs = slice(2 * u, 2 * u + 2)
                nc.sync.dma_start(out=ctx_f[:, cs, :], in_=ctxT_r[:, cs, :])
                nc.sync.dma_start(out=wk_f[:, cs, :], in_=wk_r[:, cs, :])
            wqT_f = const.tile([P, KC, C], fp8, name="wqT_f", tag="wqT_f")
            nc.sync.dma_start(out=wqT_f, in_=wqT_r)

            xT_t, x8_t = {}, {}

            def fetch8(h):
                if h < N_HT and h not in x8_t:
                    hs = slice(h * HT, (h + 1) * HT)
                    t8 = xin8.tile([P, KC, HT], fp8, tag="x8", name=f"x8_{h}")
                    nc.sync.dma_start(out=t8, in_=x8_r[:, :, hs])
                    x8_t[h] = t8

            def fetch16(h):
                if h < N_HT and h not in xT_t:
                    t16 = xin.tile([P, NCH, C], bf16, tag="xT",
                                   name=f"xT_{h}")
                    nc.sync.dma_start(
                        out=t16, in_=xT_r[:, h * NCH:(h + 1) * NCH, :])
                    xT_t[h] = t16

            fetch8(0)
            wv_f = const.tile([P, KX, C], fp8, name="wv_f", tag="wv_f")
            nc.sync.dma_start(out=wv_f, in_=wv_r)
            wo_f = const.tile([P, KC, C], fp8, name="wo_f", tag="wo_f")
            nc.sync.dma_start(out=wo_f, in_=wo_r)
            fetch8(1)
            fetch16(0)
            fetch8(2)
            fetch16(1)

            # ones (moving operand of the colsum matvec; 16-byte stride pad)
            ones2 = const.tile([P, 2, 16], fp8, name="ones2", tag="ones2")
            nc.vector.memset(ones2, 1.0)

            # PE warm-up: dummy matmuls on a memset tile fill the initial
            # DMA wait so the HAM clock gate opens before real work arrives
            warm_sb = const.tile([P, HT], bf16, name="warm_sb", tag="warm")
            nc.vector.memset(warm_sb, 0.0)
            ps_w = psum.tile([P, HT], f32, tag="mm", name="ps_warm")
            for w in range(N_WARM):
                nc.tensor.matmul(ps_w, warm_sb[:, :P], warm_sb,
                                 start=True, stop=True)

            # biases (tiny scattered DMAs on the gpsimd queue)
            bq_t = const.tile([P, KC], fp8, name="bq_t", tag="bq")
            bk_t = const.tile([P, KC], f32, name="bk_t", tag="bk")
            bv_t = const.tile([P, KC], f32, name="bv_t", tag="bv")
            with nc.allow_non_contiguous_dma(reason="tiny one-time bias loads"):
                nc.gpsimd.dma_start(out=bq_t, in_=bq_d.rearrange("(ko p) -> p ko", p=P))
                nc.gpsimd.dma_start(out=bk_t, in_=bk_d.rearrange("(ko p) -> p ko", p=P))
                nc.gpsimd.dma_start(out=bv_t, in_=bv_d.rearrange("(ko p) -> p ko", p=P))
            # bo broadcast across partitions (folded into vW exactly)
            bo_bc = const.tile([P, C], f32, name="bo_bc", tag="bo")
            bo_src = bass.AP(tensor=bo_d.tensor, offset=bo_d.offset,
                             ap=[[0, P]] + list(bo_d.ap))
            nc.gpsimd.dma_start(out=bo_bc, in_=bo_src)

            # ---------------- phase A (all fp8 DoubleRow, no transposes) ---
            # kT [128(c), KC, Lc] = (ctx Wk + bk)^T
            kT_8 = const.tile([P, KC, Lc], fp8, name="kT_8", tag="kT")
            for mc in range(KC):
                ps = psum.tile([P, Lc], f32, tag="mm", name=f"ps_k_{mc}")
                for u in range(KX // 2):
                    nc.tensor.matmul(ps,
                                     wk_f[:, 2 * u:2 * u + 2,
                                          mc * P:(mc + 1) * P],
                                     ctx_f[:, 2 * u:2 * u + 2, :],
                                     start=(u == 0), stop=(u == KX // 2 - 1),
                                     perf_mode=DR)
                nc.scalar.activation(kT_8[:, mc, :], ps, AF.Identity,
                                     scale=1.0 / WS, bias=bk_t[:, mc:mc + 1])

            # G [128(c'), KC, Lc] = Wq kT
            G_8 = const.tile([P, KC, Lc], fp8, name="G_8", tag="G")
            for mg in range(KC):
                ps = psum.tile([P, Lc], f32, tag="mmy", name=f"ps_g_{mg}")
                for u in range(KC // 2):
                    nc.tensor.matmul(ps,
                                     wqT_f[:, 2 * u:2 * u + 2,
                                           mg * P:(mg + 1) * P],
                                     kT_8[:, 2 * u:2 * u + 2, :],
                                     start=(u == 0), stop=(u == KC // 2 - 1),
                                     perf_mode=DR)
                nc.scalar.activation(G_8[:, mg, :], ps, AF.Copy,
                                     scale=1.0 / WS)

            # bqk_s [128(lc), KL] = SCALE * kT^T bq   (per-lc exp bias)
            bqk_s = const.tile([P, KL], f32, name="bqk_s", tag="bqk")
            for ml in range(KL):
                ps = psum.tile([P, HT], f32, tag="mm", name=f"ps_bq_{ml}")
                for mc in range(KC):
                    nc.tensor.matmul(ps[:, 0:1],
                                     kT_8[:, mc, ml * P:(ml + 1) * P],
                                     bq_t[:, mc:mc + 1],
                                     start=(mc == 0), stop=(mc == KC - 1))
                nc.scalar.activation(bqk_s[:, ml:ml + 1], ps[:, 0:1],
                                     AF.Identity, scale=SCALE / WS)

            # vT [128(c), KC, Lc] = (ctx Wv + bv)^T
            vT_8 = const.tile([P, KC, Lc], fp8, name="vT_8", tag="vT")
            for mc in range(KC):
                ps = psum.tile([P, Lc], f32, tag="mm", name=f"ps_vt_{mc}")
                for u in range(KX // 2):
                    nc.tensor.matmul(ps,
                                     wv_f[:, 2 * u:2 * u + 2,
                                          mc * P:(mc + 1) * P],
                                     ctx_f[:, 2 * u:2 * u + 2, :],
                                     start=(u == 0), stop=(u == KX // 2 - 1),
                                     perf_mode=DR)
                nc.scalar.activation(vT_8[:, mc, :], ps, AF.Identity,
                                     scale=1.0 / WS, bias=bv_t[:, mc:mc + 1])

            # vW [128(lc), KL, C(c_out)] = (v + bv) Wo + 1 bo^T
            vW_8 = const.tile([P, KL, C], fp8, name="vW_8", tag="vW")
            for ml in range(KL):
                ps = psum.tile([P, C], f32, tag="mmy", name=f"ps_vw_{ml}")
                for u in range(KC // 2):
                    nc.tensor.matmul(ps,
                                     vT_8[:, 2 * u:2 * u + 2,
                                          ml * P:(ml + 1) * P],
                                     wo_f[:, 2 * u:2 * u + 2, :],
                                     start=(u == 0), stop=(u == KC // 2 - 1),
                                     perf_mode=DR)
                nc.vector.scalar_tensor_tensor(
                    out=vW_8[:, ml, :], in0=ps, scalar=1.0 / WS, in1=bo_bc,
                    op0=ALU.mult, op1=ALU.add)

            # ---------------- phase B: stream over hw tiles ----------------
            def emit_yT(h, eT, last=False):
                # yT [hw, c] = (eT^T vW) / colsum + x^T.  eT chunk is the
                # stationary for BOTH the colsum matvec (N=1) and the
                # attn@V matmul (N=512), sharing weight loads.
                xT = xT_t[h]
                y_sb = yout.tile([P, NCH, C], bf16, tag="y", name=f"y_{h}")
                ps_st = psum_st.tile([P, 16], f32, tag="st", name=f"st_{h}")
                ps_y = {}
                for ch in range(NCH):
                    ps_y[ch] = psum.tile([P, C], f32, tag="mmy",
                                         name=f"ps_y_{h}_{ch}")
                    cs = slice(ch * P, (ch + 1) * P)
                    for u in range(KL // 2):
                        nc.tensor.matmul(ps_st[:, ch:ch + 1],
                                         eT[:, 2 * u:2 * u + 2, cs],
                                         ones2[:, :, 0:1],
                                         start=(u == 0), stop=(u == KL // 2 - 1),
                                         perf_mode=DR)
                        nc.tensor.matmul(ps_y[ch],
                                         eT[:, 2 * u:2 * u + 2, cs],
                                         vW_8[:, 2 * u:2 * u + 2, :],
                                         start=(u == 0), stop=(u == KL // 2 - 1),
                                         perf_mode=DR)
                rec = small.tile([P, 16], f32, tag="rec", name=f"rec_{h}")
                nc.vector.reciprocal_approx_fast(out=rec, in_=ps_st)
                for ch in range(NCH):
                    # y = ps * (1/colsum)[per-partition] + xT in one op
                    # (DVE only - GPSIMD cannot read PSUM)
                    nc.vector.scalar_tensor_tensor(
                        out=y_sb[:, ch, :], in0=ps_y[ch],
                        scalar=rec[:, ch:ch + 1], in1=xT[:, ch, :],
                        op0=ALU.mult, op1=ALU.add)
                    if last:
                        nc.sync.dma_start(
                            out=y_r[:, h * NCH + ch, :], in_=y_sb[:, ch, :])
                if not last:
                    nc.sync.dma_start(
                        out=y_r[:, h * NCH:(h + 1) * NCH, :], in_=y_sb)

            prev = None
            for h in range(N_HT):
                fetch8(h + 3)
                fetch16(h + 2)
                x_8 = x8_t[h]

                # eT [lc, hw] = exp(scale * (G^T x) + scale * kT^T bq)
                eT = work.tile([P, KL, HT], fp8, tag="eT", name=f"eT_{h}")
                for ml in range(KL):
                    ps = psum.tile([P, HT], f32, tag="mm", name=f"ps_s_{h}_{ml}")
                    for u in range(KC // 2):
                        nc.tensor.matmul(ps,
                                         G_8[:, 2 * u:2 * u + 2,
                                             ml * P:(ml + 1) * P],
                                         x_8[:, 2 * u:2 * u + 2, :],
                                         start=(u == 0), stop=(u == KC // 2 - 1),
                                         perf_mode=DR)
                    nc.scalar.activation(eT[:, ml, :], ps, AF.Exp, scale=SCALE,
                                         bias=bqk_s[:, ml:ml + 1])

                # attn@V runs one tile behind (eT fully evicted by then)
                if prev is not None:
                    emit_yT(*prev)
                prev = (h, eT)

            emit_yT(*prev, last=True)

    nc.compile()
    return nc


def _get_compiled():
    if "nc" not in _cache:
        _cache["nc"] = _build_nc()
    return _cache["nc"]


def _make_in_maps(x, context, Wq, bq, Wk, bk, Wv, bv, Wo, bo):
    x = np.asarray(x, dtype=np.float32)
    context = np.asarray(context, dtype=np.float32)
    common = {
        "wk8": np.ascontiguousarray((np.asarray(Wk, np.float32) * WS).astype(NP_FP8)),
        "wv8": np.ascontiguousarray((np.asarray(Wv, np.float32) * WS).astype(NP_FP8)),
        "wqT8": np.ascontiguousarray((np.asarray(Wq, np.float32).T * WS).astype(NP_FP8)),
        "wo8": np.ascontiguousarray((np.asarray(Wo, np.float32) * WS).astype(NP_FP8)),
        "bq8": np.ascontiguousarray((np.asarray(bq, np.float32) * WS).astype(NP_FP8)),
        "bk": np.ascontiguousarray(np.asarray(bk, dtype=np.float32)),
        "bv": np.ascontiguousarray(np.asarray(bv, dtype=np.float32)),
        "bo": np.ascontiguousarray(np.asarray(bo, dtype=np.float32)),
    }
    in_maps = []
    for b in range(B):
        m = dict(common)
        xb = x[b].reshape(C, HW)
        m["xT16"] = np.ascontiguousarray(xb.T.astype(NP_BF16))
        m["x8"] = np.ascontiguousarray(xb.astype(NP_FP8))
        m["ctxT8"] = np.ascontiguousarray(context[b].T.astype(NP_FP8))
        in_maps.append(m)
    return in_maps


def _run(in_maps, trace=False):
    from concourse.bass_utils import run_bass_kernel_spmd
    nc = _get_compiled()
    return run_bass_kernel_spmd(nc, in_maps, core_ids=list(range(N_CORES)),
                                trace=trace)


def kernel(x, context, Wq, bq, Wk, bk, Wv, bv, Wo, bo):
    in_maps = _make_in_maps(x, context, Wq, bq, Wk, bk, Wv, bv, Wo, bo)
    res = _run(in_maps, trace=False)
    out = np.stack([np.asarray(res.results[b]["yT"], dtype=np.float32)
                    .T.reshape(C, HH, WW) for b in range(B)])
    return out


# revision 8
# speedup vs baseline: 1.1700x; 1.0797x over previous
"""Cross-attention (single-head, residual) Bass/Tile kernel for Trainium2.

Problem: y = x + (softmax((x' Wq + bq)(ctx Wk + bk)^T / sqrt(C)) (ctx Wv + bv)) Wo + bo
  x: [B=8, C=512, H=64, W=64], context: [B=8, Lc=512, CTX=768]

Sharding: pure data-parallel over batch - one batch element per NeuronCore,
no collectives.

Algebraic restructuring (saves ~1/3 of the matmul work): with
  kT = (ctx Wk + bk)^T           [C, Lc]
  G  = Wq kT                     [C, Lc]   (Wq folded into the key side)
  vW = (ctx Wv + bv) Wo + 1 bo^T [Lc, C]   (Wo and bo folded into the value side)
the streaming loop per hw-tile is two matmul stages:
  simT[lc,hw] = sum_c' x[c',hw] G[c',lc]           (8 fp8 DoubleRow matmuls)
  eT   = exp(scale*simT + scale*(kT^T bq))         (bq folded into ACT bias)
  yT[hw,c] = (eT^T vW) * (1/colsum eT) + x^T       (eT is the STATIONARY)
bv/bo are exact under the fold because softmax rows sum to 1.

The attn@V matmul runs transposed (output partitions = hw) so the softmax
denominator is a per-partition scalar: colsum rides the same stationary as
free-dim-1 matmuls into a per-hw psum column, one reciprocal per tile, and
normalization + residual add fuse into a single scalar_tensor_tensor
eviction.  No broadcast matmul, no cross-engine normalization chain.

Host-side prep (layout/dtype only, no math): x ships twice - transposed
bf16 for the residual and fp8e4 as the sim moving operand; ctx arrives
pre-transposed in fp8; Wq pre-transposed; weights fp8 scaled by 32 (raw
std 0.02 is fp8 subnormal; evictions unscale).  A block of dummy warm-up
matmuls fills the initial DMA wait so the PE HAM clock gate opens
(1.2 -> 2.4 GHz) before real work arrives.  y is written [HW, C] bf16 and
transposed back on the host.
"""

import numpy as np
import ml_dtypes

B = 8
C = 512
CTX = 768
Lc = 512
HH = 64
WW = 64
HW = HH * WW          # 4096
N_CORES = 8
P = 128
HT = 512              # hw tile (free-dim) width
N_HT = HW // HT       # 8
NCH = HT // P         # 4 hw chunks per tile
KC = C // P           # 4
KX = CTX // P         # 6
KL = Lc // P          # 4
SCALE = float(C) ** -0.5
WS = 32.0             # host-side fp8 weight scaling
N_WARM = 12           # PE warm-up matmuls during the initial DMA wait

NP_BF16 = ml_dtypes.bfloat16
NP_FP8 = ml_dtypes.float8_e4m3

_cache = {}


def _build_nc():
    import concourse.mybir as mybir
    import concourse.bass as bass
    import concourse.tile as tile
    from concourse import bacc

    f32 = mybir.dt.float32
    bf16 = mybir.dt.bfloat16
    fp8 = mybir.dt.float8e4
    AF = mybir.ActivationFunctionType
    ALU = mybir.AluOpType
    DR = mybir.MatmulPerfMode.DoubleRow

    nc = bacc.Bacc("TRN2", target_bir_lowering=False, debug=False,
                   num_devices=N_CORES)

    xT_d = nc.dram_tensor("xT16", [HW, C], bf16, kind="ExternalInput").ap()
    x8_d = nc.dram_tensor("x8", [C, HW], fp8, kind="ExternalInput").ap()
    ctxT_d = nc.dram_tensor("ctxT8", [CTX, Lc], fp8, kind="ExternalInput").ap()
    wk_d = nc.dram_tensor("wk8", [CTX, C], fp8, kind="ExternalInput").ap()
    wv_d = nc.dram_tensor("wv8", [CTX, C], fp8, kind="ExternalInput").ap()
    wqT_d = nc.dram_tensor("wqT8", [C, C], fp8, kind="ExternalInput").ap()
    wo_d = nc.dram_tensor("wo8", [C, C], fp8, kind="ExternalInput").ap()
    bq_d = nc.dram_tensor("bq8", [C], fp8, kind="ExternalInput").ap()
    bk_d = nc.dram_tensor("bk", [C], f32, kind="ExternalInput").ap()
    bv_d = nc.dram_tensor("bv", [C], f32, kind="ExternalInput").ap()
    bo_d = nc.dram_tensor("bo", [C], f32, kind="ExternalInput").ap()
    y_d = nc.dram_tensor("yT", [HW, C], bf16, kind="ExternalOutput").ap()

    xT_r = xT_d.rearrange("(hh p) c -> p hh c", p=P)        # [128, 32, 512]
    y_r = y_d.rearrange("(hh p) c -> p hh c", p=P)
    x8_r = x8_d.rearrange("(ko p) hw -> p ko hw", p=P)      # [128, 4, 4096]
    ctxT_r = ctxT_d.rearrange("(ko p) lc -> p ko lc", p=P)  # [128, 6, 512]
    wk_r = wk_d.rearrange("(ko p) c -> p ko c", p=P)        # [128, 6, 512]
    wv_r = wv_d.rearrange("(ko p) c -> p ko c", p=P)
    wqT_r = wqT_d.rearrange("(ko p) c -> p ko c", p=P)      # [128, 4, 512]
    wo_r = wo_d.rearrange("(ko p) c -> p ko c", p=P)

    with tile.TileContext(nc) as tc:
        with (
            tc.tile_pool(name="const", bufs=1) as const,
            tc.tile_pool(name="xin", bufs=4) as xin,
            tc.tile_pool(name="xin8", bufs=4) as xin8,
            tc.tile_pool(name="work", bufs=3) as work,
            tc.tile_pool(name="yout", bufs=2) as yout,
            tc.tile_pool(name="small", bufs=3) as small,
            tc.tile_pool(name="psum", bufs=3, space="PSUM") as psum,
            tc.tile_pool(name="psum_st", bufs=2, space="PSUM") as psum_st,
        ):
            # ---------------- DMAs (ordered by when the PE needs them) -----
            ctx_f = const.tile([P, KX, Lc], fp8, name="ctx_f", tag="ctx_f")
            wk_f = const.tile([P, KX, C], fp8, name="wk_f", tag="wk_f")
            for u in range(KX // 2):
                cs = slice(2 * u, 2 * u + 2)
                nc.sync.dma_start(out=ctx_f[:, cs, :], in_=ctxT_r[:, cs, :])
                nc.sync.dma_start(out=wk_f[:, cs, :], in_=wk_r[:, cs, :])
            wqT_f = const.tile([P, KC, C], fp8, name="wqT_f", tag="wqT_f")
            nc.sync.dma_start(out=wqT_f, in_=wqT_r)

            xT_t, x8_t = {}, {}

            def fetch8(h):
                if h < N_HT and h not in x8_t:
                    hs = slice(h * HT, (h + 1) * HT)
                    t8 = xin8.tile([P, KC, HT], fp8, tag="x8", name=f"x8_{h}")
                    nc.sync.dma_start(out=t8, in_=x8_r[:, :, hs])
                    x8_t[h] = t8

            def fetch16(h):
                if h < N_HT and h not in xT_t:
                    t16 = xin.tile([P, NCH, C], bf16, tag="xT",
                                   name=f"xT_{h}")
                    nc.sync.dma_start(
                        out=t16, in_=xT_r[:, h * NCH:(h + 1) * NCH, :])
                    xT_t[h] = t16

            fetch8(0)
            wv_f = const.tile([P, KX, C], fp8, name="wv_f", tag="wv_f")
            nc.sync.dma_start(out=wv_f, in_=wv_r)
            wo_f = const.tile([P, KC, C], fp8, name="wo_f", tag="wo_f")
            nc.sync.dma_start(out=wo_f, in_=wo_r)
            fetch8(1)
            fetch16(0)
            fetch8(2)
            fetch16(1)

            # ones (moving operand of the colsum matvec; 16-byte stride pad)
            ones2 = const.tile([P, 2, 16], fp8, name="ones2", tag="ones2")
            nc.vector.memset(ones2, 1.0)

            # PE warm-up: dummy matmuls on a memset tile fill the initial
            # DMA wait so the HAM clock gate opens before real work arrives
            warm_sb = const.tile([P, HT], bf16, name="warm_sb", tag="warm")
            nc.vector.memset(warm_sb, 0.0)
            ps_w = psum.tile([P, HT], f32, tag="mm", name="ps_warm")
            for w in range(N_WARM):
                nc.tensor.matmul(ps_w, warm_sb[:, :P], warm_sb,
                                 start=True, stop=True)

            # biases (tiny scattered DMAs on the gpsimd queue)
            bq_t = const.tile([P, KC], fp8, name="bq_t", tag="bq")
            bk_t = const.tile([P, KC], f32, name="bk_t", tag="bk")
            bv_t = const.tile([P, KC], f32, name="bv_t", tag="bv")
            with nc.allow_non_contiguous_dma(reason="tiny one-time bias loads"):
                nc.gpsimd.dma_start(out=bq_t, in_=bq_d.rearrange("(ko p) -> p ko", p=P))
                nc.gpsimd.dma_start(out=bk_t, in_=bk_d.rearrange("(ko p) -> p ko", p=P))
                nc.gpsimd.dma_start(out=bv_t, in_=bv_d.rearrange("(ko p) -> p ko", p=P))
            # bo broadcast across partitions (folded into vW exactly)
            bo_bc = const.tile([P, C], f32, name="bo_bc", tag="bo")
            bo_src = bass.AP(tensor=bo_d.tensor, offset=bo_d.offset,
                             ap=[[0, P]] + list(bo_d.ap))
            nc.gpsimd.dma_start(out=bo_bc, in_=bo_src)

            # ---------------- phase A (all fp8 DoubleRow, no transposes) ---
            # kT [128(c), KC, Lc] = (ctx Wk + bk)^T
            kT_8 = const.tile([P, KC, Lc], fp8, name="kT_8", tag="kT")
            for mc in range(KC):
                ps = psum.tile([P, Lc], f32, tag="mm", name=f"ps_k_{mc}")
                for u in range(KX // 2):
                    nc.tensor.matmul(ps,
                                     wk_f[:, 2 * u:2 * u + 2,
                                          mc * P:(mc + 1) * P],
                                     ctx_f[:, 2 * u:2 * u + 2, :],
                                     start=(u == 0), stop=(u == KX // 2 - 1),
                                     perf_mode=DR)
                nc.scalar.activation(kT_8[:, mc, :], ps, AF.Identity,
                                     scale=1.0 / WS, bias=bk_t[:, mc:mc + 1])

            # G [128(c'), KC, Lc] = Wq kT
            G_8 = const.tile([P, KC, Lc], fp8, name="G_8", tag="G")
            for mg in range(KC):
                ps = psum.tile([P, Lc], f32, tag="mmy", name=f"ps_g_{mg}")
                for u in range(KC // 2):
                    nc.tensor.matmul(ps,
                                     wqT_f[:, 2 * u:2 * u + 2,
                                           mg * P:(mg + 1) * P],
                                     kT_8[:, 2 * u:2 * u + 2, :],
                                     start=(u == 0), stop=(u == KC // 2 - 1),
                                     perf_mode=DR)
                nc.scalar.activation(G_8[:, mg, :], ps, AF.Copy,
                                     scale=1.0 / WS)

            # bqk_s [128(lc), KL] = SCALE * kT^T bq   (per-lc exp bias)
            bqk_s = const.tile([P, KL], f32, name="bqk_s", tag="bqk")
            for ml in range(KL):
                ps = psum.tile([P, HT], f32, tag="mm", name=f"ps_bq_{ml}")
                for mc in range(KC):
                    nc.tensor.matmul(ps[:, 0:1],
                                     kT_8[:, mc, ml * P:(ml + 1) * P],
                                     bq_t[:, mc:mc + 1],
                                     start=(mc == 0), stop=(mc == KC - 1))
                nc.scalar.activation(bqk_s[:, ml:ml + 1], ps[:, 0:1],
                                     AF.Identity, scale=SCALE / WS)

            # sim tile 0 runs here, before the V-side precompute, so the PE
            # fills the gap while ACT drains the G evictions
            def emit_sim(h):
                x_8 = x8_t[h]
                eT = work.tile([P, KL, HT], fp8, tag="eT", name=f"eT_{h}")
                for ml in range(KL):
                    ps = psum.tile([P, HT], f32, tag="mm", name=f"ps_s_{h}_{ml}")
                    for u in range(KC // 2):
                        nc.tensor.matmul(ps,
                                         G_8[:, 2 * u:2 * u + 2,
                                             ml * P:(ml + 1) * P],
                                         x_8[:, 2 * u:2 * u + 2, :],
                                         start=(u == 0), stop=(u == KC // 2 - 1),
                                         perf_mode=DR)
                    nc.scalar.activation(eT[:, ml, :], ps, AF.Exp, scale=SCALE,
                                         bias=bqk_s[:, ml:ml + 1])
                return eT

            eT0 = emit_sim(0)

            # vT [128(c), KC, Lc] = (ctx Wv + bv)^T
            vT_8 = const.tile([P, KC, Lc], fp8, name="vT_8", tag="vT")
            for mc in range(KC):
                ps = psum.tile([P, Lc], f32, tag="mm", name=f"ps_vt_{mc}")
                for u in range(KX // 2):
                    nc.tensor.matmul(ps,
                                     wv_f[:, 2 * u:2 * u + 2,
                                          mc * P:(mc + 1) * P],
                                     ctx_f[:, 2 * u:2 * u + 2, :],
                                     start=(u == 0), stop=(u == KX // 2 - 1),
                                     perf_mode=DR)
                nc.scalar.activation(vT_8[:, mc, :], ps, AF.Identity,
                                     scale=1.0 / WS, bias=bv_t[:, mc:mc + 1])

            # vW [128(lc), KL, C(c_out)] = (v + bv) Wo + 1 bo^T
            vW_8 = const.tile([P, KL, C], fp8, name="vW_8", tag="vW")
            for ml in range(KL):
                ps = psum.tile([P, C], f32, tag="mmy", name=f"ps_vw_{ml}")
                for u in range(KC // 2):
                    nc.tensor.matmul(ps,
                                     vT_8[:, 2 * u:2 * u + 2,
                                          ml * P:(ml + 1) * P],
                                     wo_f[:, 2 * u:2 * u + 2, :],
                                     start=(u == 0), stop=(u == KC // 2 - 1),
                                     perf_mode=DR)
                nc.vector.scalar_tensor_tensor(
                    out=vW_8[:, ml, :], in0=ps, scalar=1.0 / WS, in1=bo_bc,
                    op0=ALU.mult, op1=ALU.add)

            # ---------------- phase B: stream over hw tiles ----------------
            def emit_yT(h, eT, last=False):
                # yT [hw, c] = (eT^T vW) / colsum + x^T.  eT chunk is the
                # stationary for BOTH the colsum matvec (N=1) and the
                # attn@V matmul (N=512), sharing weight loads.
                xT = xT_t[h]
                y_sb = yout.tile([P, NCH, C], bf16, tag="y", name=f"y_{h}")

                def chunk_mms(ch, ps_st, st_col):
                    ps_y = psum.tile([P, C], f32, tag="mmy",
                                     name=f"ps_y_{h}_{ch}")
                    cs = slice(ch * P, (ch + 1) * P)
                    for u in range(KL // 2):
                        nc.tensor.matmul(ps_st[:, st_col:st_col + 1],
                                         eT[:, 2 * u:2 * u + 2, cs],
                                         ones2[:, :, 0:1],
                                         start=(u == 0), stop=(u == KL // 2 - 1),
                                         perf_mode=DR)
                        nc.tensor.matmul(ps_y,
                                         eT[:, 2 * u:2 * u + 2, cs],
                                         vW_8[:, 2 * u:2 * u + 2, :],
                                         start=(u == 0), stop=(u == KL // 2 - 1),
                                         perf_mode=DR)
                    return ps_y

                def evict(ch, ps_y, rec, rec_col):
                    # y = ps * (1/colsum)[per-partition] + xT in one op
                    # (DVE only - GPSIMD cannot read PSUM)
                    nc.vector.scalar_tensor_tensor(
                        out=y_sb[:, ch, :], in0=ps_y,
                        scalar=rec[:, rec_col:rec_col + 1], in1=xT[:, ch, :],
                        op0=ALU.mult, op1=ALU.add)

                if not last:
                    ps_st = psum_st.tile([P, 16], f32, tag="st", name=f"st_{h}")
                    ps_y = [chunk_mms(ch, ps_st, ch) for ch in range(NCH)]
                    rec = small.tile([P, 16], f32, tag="rec", name=f"rec_{h}")
                    nc.vector.reciprocal_approx_fast(out=rec, in_=ps_st)
                    for ch in range(NCH):
                        evict(ch, ps_y[ch], rec, ch)
                    nc.sync.dma_start(
                        out=y_r[:, h * NCH:(h + 1) * NCH, :], in_=y_sb)
                else:
                    # drain tail: pipeline per chunk (own psum column group,
                    # reciprocal, eviction, DMA) so the DVE/DMA overlap the
                    # remaining chunks' matmuls
                    for ch in range(NCH):
                        ps_st = psum_st.tile([P, 16], f32, tag="st",
                                             name=f"st_{h}_{ch}")
                        ps_y = chunk_mms(ch, ps_st, 0)
                        rec = small.tile([P, 16], f32, tag="rec",
                                         name=f"rec_{h}_{ch}")
                        nc.vector.reciprocal_approx_fast(
                            out=rec[:, 0:1], in_=ps_st[:, 0:1])
                        evict(ch, ps_y, rec, 0)
                        nc.sync.dma_start(
                            out=y_r[:, h * NCH + ch, :], in_=y_sb[:, ch, :])

            prev = (0, eT0)
            for h in range(1, N_HT):
                fetch8(h + 2)
                fetch16(h + 1)
                eT = emit_sim(h)
                # attn@V runs one tile behind (eT fully evicted by then)
                emit_yT(*prev)
                prev = (h, eT)

            emit_yT(*prev, last=True)

    nc.compile()
    return nc


def _get_compiled():
    if "nc" not in _cache:
        _cache["nc"] = _build_nc()
    return _cache["nc"]


def _make_in_maps(x, context, Wq, bq, Wk, bk, Wv, bv, Wo, bo):
    x = np.asarray(x, dtype=np.float32)
    context = np.asarray(context, dtype=np.float32)
    common = {
        "wk8": np.ascontiguousarray((np.asarray(Wk, np.float32) * WS).astype(NP_FP8)),
        "wv8": np.ascontiguousarray((np.asarray(Wv, np.float32) * WS).astype(NP_FP8)),
        "wqT8": np.ascontiguousarray((np.asarray(Wq, np.float32).T * WS).astype(NP_FP8)),
        "wo8": np.ascontiguousarray((np.asarray(Wo, np.float32) * WS).astype(NP_FP8)),
        "bq8": np.ascontiguousarray((np.asarray(bq, np.float32) * WS).astype(NP_FP8)),
        "bk": np.ascontiguousarray(np.asarray(bk, dtype=np.float32)),
        "bv": np.ascontiguousarray(np.asarray(bv, dtype=np.float32)),
        "bo": np.ascontiguousarray(np.asarray(bo, dtype=np.float32)),
    }
    in_maps = []
    for b in range(B):
        m = dict(common)
        xb = x[b].reshape(C, HW)
        m["xT16"] = np.ascontiguousarray(xb.T.astype(NP_BF16))
        m["x8"] = np.ascontiguousarray(xb.astype(NP_FP8))
        m["ctxT8"] = np.ascontiguousarray(context[b].T.astype(NP_FP8))
        in_maps.append(m)
    return in_maps


def _run(in_maps, trace=False):
    from concourse.bass_utils import run_bass_kernel_spmd
    nc = _get_compiled()
    return run_bass_kernel_spmd(nc, in_maps, core_ids=list(range(N_CORES)),
                                trace=trace)


def kernel(x, context, Wq, bq, Wk, bk, Wv, bv, Wo, bo):
    in_maps = _make_in_maps(x, context, Wq, bq, Wk, bk, Wv, bv, Wo, bo)
    res = _run(in_maps, trace=False)
    out = np.stack([np.asarray(res.results[b]["yT"], dtype=np.float32)
                    .T.reshape(C, HH, WW) for b in range(B)])
    return out
